# revision 55
# baseline (speedup 1.0000x reference)
"""Bass/Trainium2 kernel for nn_Net_58832462020811 (DiffPool GNN on a radius graph).

Key algebra (exact for this problem's inputs): the radius threshold (5e6) vastly
exceeds any pairwise distance of the N(0,1) inputs, so adj == ones - eye and
adj @ x == colsum(x) - x with degree 2047. Every DenseSAGEConv therefore
collapses to a single dense matmul with folded weights/bias:
    dense_sage(x) = x @ (w_root - w_rel/2047) + [b_rel + (colsum(x) @ w_rel)/2047]
The only heavy compute left is embed1: r1 = relu(H @ W1f + b1f) with
H [2048,512], W1f [512,512], followed by BatchNorm (train-mode stats over the
2048 rows) and the DiffPool contraction P = s^T @ r1 with s [2048,2].

Device work (8 cores, row-sharded 256 rows/core, zero collectives):
  z = H_rows @ W1f + b1f ; r1 = relu(z) ; per-core partials of
  [P0, P1, colsum(r1), colsum(r1^2)] -> out [4,512] per core.
All matmul operands are bf16 (single-pass PE, FWL-eligible; f32 PSUM
accumulate); measured end-to-end rel err is unchanged vs all-f32.
BatchNorm is applied after the fact via the folded form
  s^T @ BN(r1) = (s^T @ r1) * a + colsum(s) (x) (beta - a*mu),  a = gamma/rstd,
so no cross-core stat exchange is needed on device.

Host work (tiny, O(N*C) with C=2): the pool head (H @ Wpf is 2M MACs), softmax
assignments s, link/entropy losses, the pooled 2x2 graph, embed3 on [2,512],
and the final MLP + log_softmax.
"""

import ml_dtypes
import numpy as np

import concourse.bass as bass
import concourse.tile as tile
from concourse import mybir
from concourse.bass_utils import run_bass_kernel_spmd

N = 2048
L = 512
NCORES = 8
M = N // NCORES          # rows per core
DEG = float(N - 1)       # 2047, degree of every node
EPS = 1e-5

F32 = mybir.dt.float32
BF16 = mybir.dt.bfloat16


KT = L // 128   # 4 k-tiles
MT = M // 128   # 2 row-tiles per core
# Inputs (per core, bf16 matmul path):
#   hw  [128, KT*(M+L)]: chunk k at [:, k*(M+L):...]; within a chunk,
#                        cols 0:M = H_rows^T k-tile (lhsT), M:M+L = W1f k-tile
#   sab [128, MT*4]:    sab[p, t*4+j] = sa[t*128+p, j], sa cols [s0, s1, 1, 0]
#   bo  [1, 128+L]:     ones[128] ++ b1f[512]  (rank-1 bias matmul operands)


def _split_waits(nc, cap=1):
    """walrus codegen in this env rejects >1 sync wait per instruction
    ("Too many sync wait commands"); hoist extras onto same-engine NoOps
    placed immediately before (sequencer order makes this equivalent)."""
    for blk in nc.main_func.blocks:
        lst = blk.instructions
        for ins in list(lst):
            si = ins.sync_info
            if si is None or len(si.on_wait) <= cap:
                continue
            if ins.engine == mybir.EngineType.Unassigned:
                continue
            extra = list(si.on_wait[cap:])
            keep = list(si.on_wait[:cap])
            idx = lst.index(ins)
            for w in extra:
                nop = mybir.InstNoOp(
                    name=nc.get_next_instruction_name(),
                    ins=[], outs=[],
                    sync_info=mybir.SyncInfo(on_wait=[w], on_update=[]),
                )
                nop.engine = ins.engine
                lst.insert(idx, nop)
                idx += 1
            ins.sync_info = mybir.SyncInfo(on_wait=keep,
                                           on_update=list(si.on_update))


def _build_nc(reps=1, loop=False):
    nc = bass.Bass()
    hw = nc.dram_tensor("hw", [128, KT * (M + L)], BF16, kind="ExternalInput")
    sab = nc.dram_tensor("sab", [128, MT * 4], BF16, kind="ExternalInput")
    bo = nc.dram_tensor("bo", [1, 128 + L], BF16, kind="ExternalInput")
    out = nc.dram_tensor("out", [4, 2, L], F32, kind="ExternalOutput")

    with tile.TileContext(nc) as tc:
        with (
            tc.tile_pool(name="sb", bufs=1) as sb,
            tc.tile_pool(name="act", bufs=2) as act,
            tc.tile_pool(name="psz", bufs=2, space="PSUM") as psz_pool,
            tc.tile_pool(name="pacc", bufs=1, space="PSUM") as pacc,
        ):
            def body():
                big = sb.tile([128, KT * (M + L)], BF16)
                sa_sb = sb.tile([128, MT * 4], BF16)
                bo_sb = sb.tile([1, 128 + L], BF16)
                # small operands first (bias mm unblocks immediately); hw as
                # chunk0 + rest so the first z-matmuls overlap the DMA tail
                nc.sync.dma_start(out=bo_sb, in_=bo[:, :])
                nc.sync.dma_start(out=sa_sb, in_=sab[:, :])
                nc.sync.dma_start(out=big[:, 0:M + L], in_=hw[:, 0:M + L])
                nc.sync.dma_start(out=big[:, M + L:], in_=hw[:, M + L:])
                ck = [big[:, k * (M + L):(k + 1) * (M + L)] for k in range(KT)]

                psP = pacc.tile([4, L], F32)   # rows: P0, P1, colsum(r1), 0
                psQ = pacc.tile([4, L], F32)   # rows: *, *, colsum(r1^2), 0

                for mt in range(MT):
                    psz = psz_pool.tile([128, L], F32)
                    # bias via rank-1 matmul: ones[128,1] @ b1f[1,512]
                    nc.tensor.matmul(psz, bo_sb[0:1, 0:128],
                                     bo_sb[0:1, 128:128 + L],
                                     start=True, stop=False)
                    for k in range(KT):
                        nc.tensor.matmul(
                            psz,
                            ck[k][:, mt * 128:(mt + 1) * 128],
                            ck[k][:, M:M + L],
                            start=False,
                            stop=(k == KT - 1),
                        )
                    r1 = act.tile([128, L], BF16)
                    nc.scalar.activation(out=r1, in_=psz,
                                         func=mybir.ActivationFunctionType.Relu)
                    r1sq = act.tile([128, L], BF16)
                    nc.vector.tensor_mul(r1sq, r1, r1)
                    nc.tensor.matmul(psP, sa_sb[:, mt * 4:(mt + 1) * 4], r1,
                                     start=(mt == 0), stop=(mt == MT - 1))
                    nc.tensor.matmul(psQ, sa_sb[:, mt * 4:(mt + 1) * 4], r1sq,
                                     start=(mt == 0), stop=(mt == MT - 1))

                outbuf = sb.tile([4, 2, L], F32)
                nc.scalar.activation(out=outbuf[:, 0, :], in_=psP,
                                     func=mybir.ActivationFunctionType.Copy)
                nc.vector.tensor_copy(outbuf[:, 1, :], psQ)
                nc.sync.dma_start(out=out[:, :, :], in_=outbuf)

            if loop:
                with tc.For_i(0, reps, 1):
                    body()
            else:
                for _ in range(reps):
                    body()
    _split_waits(nc)
    return nc


_NC = {}
LAST_IN_MAPS = None


def _get_nc(reps=1, loop=False):
    key = (reps, loop)
    if key not in _NC:
        _NC[key] = _build_nc(reps, loop)
    return _NC[key]


def _batchnorm_rows(x, gamma, beta):
    mu = x.mean(axis=0)
    var = x.var(axis=0)
    return (x - mu) / np.sqrt(var + EPS) * gamma + beta


def kernel(**inputs):
    f8 = np.float64
    H = np.asarray(inputs["H"], np.float32)
    Hd = H.astype(f8)
    S = Hd.sum(axis=0)                                   # colsum(H) [512]

    # ---------------- host: pool head -> assignments s ----------------
    pw_rel = inputs["pool_w_rel"].astype(f8)
    Wp = inputs["pool_w_root"].astype(f8) - pw_rel / DEG
    bp = inputs["pool_b_rel"].astype(f8) + (S @ pw_rel) / DEG
    rp = np.maximum(Hd @ Wp + bp, 0.0)                   # [2048, 2]
    xp = _batchnorm_rows(rp, inputs["pool_gamma"].astype(f8),
                         inputs["pool_beta"].astype(f8))
    zl = np.maximum(xp @ inputs["pool_w_lin"].astype(f8)
                    + inputs["pool_b_lin"].astype(f8), 0.0)
    zs = zl - zl.max(axis=-1, keepdims=True)
    es = np.exp(zs)
    s = es / es.sum(axis=-1, keepdims=True)              # [2048, 2]

    cs = s.sum(axis=0)                                   # [2]
    G = s.T @ s                                          # [2, 2]
    ent = float((-s * np.log(s + 1e-15)).sum(axis=-1).mean())
    # ||(ones - eye) - s s^T||_F^2 without materializing NxN:
    fro2 = N * N - N - 2.0 * float(cs @ cs) + 2.0 * float(np.trace(G)) \
        + float((G * G).sum())
    l1 = float(np.sqrt(max(fro2, 0.0)) / (N * N))

    # ---------------- device: embed1 + diffpool contraction ----------------
    e_rel = inputs["embed1_w_rel"].astype(f8)
    W1 = (inputs["embed1_w_root"].astype(f8) - e_rel / DEG).astype(np.float32)
    b1f = (inputs["embed1_b_rel"].astype(f8) + (S @ e_rel) / DEG).astype(np.float32)
    bop = np.zeros((1, 128 + L), ml_dtypes.bfloat16)
    bop[0, :128] = 1.0
    bop[0, 128:] = b1f

    HT = np.ascontiguousarray(H.T)                       # [512, 2048]
    sa = np.zeros((N, 4), np.float32)
    sa[:, 0:2] = s.astype(np.float32)
    sa[:, 2] = 1.0
    W1k = W1.reshape(KT, 128, L)                         # [4,128,512]

    in_maps = []
    for i in range(NCORES):
        hts = HT[:, i * M:(i + 1) * M].reshape(KT, 128, M)
        hwp = np.ascontiguousarray(
            np.concatenate([hts, W1k], axis=2).transpose(1, 0, 2)
            .reshape(128, KT * (M + L))).astype(ml_dtypes.bfloat16)
        sap = sa[i * M:(i + 1) * M].reshape(MT, 128, 4)
        sap = np.ascontiguousarray(
            sap.transpose(1, 0, 2).reshape(128, MT * 4)).astype(ml_dtypes.bfloat16)
        in_maps.append({"hw": hwp, "sab": sap, "bo": bop})

    global LAST_IN_MAPS
    LAST_IN_MAPS = in_maps
    res = run_bass_kernel_spmd(_get_nc(), in_maps, list(range(NCORES)))
    parts = np.stack([res.results[i]["out"] for i in range(NCORES)]).astype(f8)
    tot = parts.sum(axis=0)                              # [4, 2, 512]
    P = tot[0:2, 0, :]                                   # s^T @ r1
    mu1 = tot[2, 0, :] / N
    var1 = tot[2, 1, :] / N - mu1 * mu1
    a1 = inputs["embed1_gamma"].astype(f8) / np.sqrt(var1 + EPS)
    c1 = inputs["embed1_beta"].astype(f8) - a1 * mu1
    x = P * a1[None, :] + cs[:, None] * c1[None, :]      # pooled features [2,512]
    adj = np.outer(cs, cs) - G                           # pooled adjacency [2,2]

    # ---------------- host: embed3 + classifier ----------------
    deg3 = np.clip(adj.sum(axis=-1, keepdims=True), 1.0, None)
    agg = (adj @ x) / deg3
    z3 = agg @ inputs["embed3_w_rel"].astype(f8) + inputs["embed3_b_rel"].astype(f8) \
        + x @ inputs["embed3_w_root"].astype(f8)
    x3 = _batchnorm_rows(np.maximum(z3, 0.0), inputs["embed3_gamma"].astype(f8),
                         inputs["embed3_beta"].astype(f8))
    xm = x3.mean(axis=0)                                 # [512]
    h1 = np.maximum(xm @ inputs["w1"].astype(f8) + inputs["b1"].astype(f8), 0.0)
    z = h1 @ inputs["w2"].astype(f8) + inputs["b2"].astype(f8)
    zst = z - z.max()
    lp = zst - np.log(np.exp(zst).sum())
    return (lp.astype(np.float32).reshape(1, 2),
            np.float32(l1), np.float32(ent))


# revision 57
# speedup vs baseline: 1.0076x; 1.0076x over previous
"""Bass/Trainium2 kernel for nn_Net_58832462020811 (DiffPool GNN on a radius graph).

Key algebra (exact for this problem's inputs): the radius threshold (5e6) vastly
exceeds any pairwise distance of the N(0,1) inputs, so adj == ones - eye and
adj @ x == colsum(x) - x with degree 2047. Every DenseSAGEConv therefore
collapses to a single dense matmul with folded weights/bias:
    dense_sage(x) = x @ (w_root - w_rel/2047) + [b_rel + (colsum(x) @ w_rel)/2047]
The only heavy compute left is embed1: r1 = relu(H @ W1f + b1f) with
H [2048,512], W1f [512,512], followed by BatchNorm (train-mode stats over the
2048 rows) and the DiffPool contraction P = s^T @ r1 with s [2048,2].

Device work (8 cores, row-sharded 256 rows/core, zero collectives):
  z = H_rows @ W1f + b1f ; r1 = relu(z) ; per-core partials of
  [P0, P1, colsum(r1), colsum(r1^2)] -> out [4,512] per core.
All matmul operands are bf16 (single-pass PE, FWL-eligible; f32 PSUM
accumulate); measured end-to-end rel err is unchanged vs all-f32.
BatchNorm is applied after the fact via the folded form
  s^T @ BN(r1) = (s^T @ r1) * a + colsum(s) (x) (beta - a*mu),  a = gamma/rstd,
so no cross-core stat exchange is needed on device.

Host work (tiny, O(N*C) with C=2): the pool head (H @ Wpf is 2M MACs), softmax
assignments s, link/entropy losses, the pooled 2x2 graph, embed3 on [2,512],
and the final MLP + log_softmax.
"""

import ml_dtypes
import numpy as np

import concourse.bass as bass
import concourse.tile as tile
from concourse import mybir
from concourse.bass_utils import run_bass_kernel_spmd

N = 2048
L = 512
NCORES = 8
M = N // NCORES          # rows per core
DEG = float(N - 1)       # 2047, degree of every node
EPS = 1e-5

F32 = mybir.dt.float32
BF16 = mybir.dt.bfloat16


KT = L // 128   # 4 k-tiles
MT = M // 128   # 2 row-tiles per core
# Inputs (per core, bf16 matmul path):
#   hw  [128, KT*(M+L)]: chunk k at [:, k*(M+L):...]; within a chunk,
#                        cols 0:M = H_rows^T k-tile (lhsT), M:M+L = W1f k-tile
#   sab [128, MT*4]:    sab[p, t*4+j] = sa[t*128+p, j], sa cols [s0, s1, 1, 0]
#   bo  [1, 128+L]:     ones[128] ++ b1f[512]  (rank-1 bias matmul operands)


def _split_waits(nc, cap=1):
    """walrus codegen in this env rejects >1 sync wait per instruction
    ("Too many sync wait commands"); hoist extras onto same-engine NoOps
    placed immediately before (sequencer order makes this equivalent)."""
    for blk in nc.main_func.blocks:
        lst = blk.instructions
        for ins in list(lst):
            si = ins.sync_info
            if si is None or len(si.on_wait) <= cap:
                continue
            if ins.engine == mybir.EngineType.Unassigned:
                continue
            extra = list(si.on_wait[cap:])
            keep = list(si.on_wait[:cap])
            idx = lst.index(ins)
            for w in extra:
                nop = mybir.InstNoOp(
                    name=nc.get_next_instruction_name(),
                    ins=[], outs=[],
                    sync_info=mybir.SyncInfo(on_wait=[w], on_update=[]),
                )
                nop.engine = ins.engine
                lst.insert(idx, nop)
                idx += 1
            ins.sync_info = mybir.SyncInfo(on_wait=keep,
                                           on_update=list(si.on_update))


def _build_nc(reps=1, loop=False):
    nc = bass.Bass()
    hw = nc.dram_tensor("hw", [128, KT * (M + L)], BF16, kind="ExternalInput")
    sab = nc.dram_tensor("sab", [128, MT * 4], BF16, kind="ExternalInput")
    bo = nc.dram_tensor("bo", [1, 128 + L], BF16, kind="ExternalInput")
    out = nc.dram_tensor("out", [4, 2, L], F32, kind="ExternalOutput")

    with tile.TileContext(nc) as tc:
        with (
            tc.tile_pool(name="sb", bufs=1) as sb,
            tc.tile_pool(name="act", bufs=2) as act,
            tc.tile_pool(name="psz", bufs=2, space="PSUM") as psz_pool,
            tc.tile_pool(name="pacc", bufs=1, space="PSUM") as pacc,
        ):
            def body():
                big = sb.tile([128, KT * (M + L)], BF16)
                sa_sb = sb.tile([128, MT * 4], BF16)
                bo_sb = sb.tile([1, 128 + L], BF16)
                # small operands first (bias mm unblocks immediately); hw split
                # across two DGE queues (partition halves -> parallel SDMA) and
                # chunk0/rest so the first z-matmuls overlap the DMA tail
                nc.sync.dma_start(out=bo_sb, in_=bo[:, :])
                nc.sync.dma_start(out=sa_sb, in_=sab[:, :])
                c0 = M + L
                nc.sync.dma_start(out=big[0:64, 0:c0], in_=hw[0:64, 0:c0])
                nc.scalar.dma_start(out=big[64:128, 0:c0], in_=hw[64:128, 0:c0])
                nc.sync.dma_start(out=big[0:64, c0:], in_=hw[0:64, c0:])
                nc.scalar.dma_start(out=big[64:128, c0:], in_=hw[64:128, c0:])
                ck = [big[:, k * (M + L):(k + 1) * (M + L)] for k in range(KT)]

                psP = pacc.tile([4, L], F32)   # rows: P0, P1, colsum(r1), 0
                psQ = pacc.tile([4, L], F32)   # rows: *, *, colsum(r1^2), 0

                for mt in range(MT):
                    psz = psz_pool.tile([128, L], F32)
                    # bias via rank-1 matmul: ones[128,1] @ b1f[1,512]
                    nc.tensor.matmul(psz, bo_sb[0:1, 0:128],
                                     bo_sb[0:1, 128:128 + L],
                                     start=True, stop=False)
                    for k in range(KT):
                        nc.tensor.matmul(
                            psz,
                            ck[k][:, mt * 128:(mt + 1) * 128],
                            ck[k][:, M:M + L],
                            start=False,
                            stop=(k == KT - 1),
                        )
                    r1 = act.tile([128, L], BF16)
                    nc.scalar.activation(out=r1, in_=psz,
                                         func=mybir.ActivationFunctionType.Relu)
                    r1sq = act.tile([128, L], BF16)
                    nc.vector.tensor_mul(r1sq, r1, r1)
                    nc.tensor.matmul(psP, sa_sb[:, mt * 4:(mt + 1) * 4], r1,
                                     start=(mt == 0), stop=(mt == MT - 1))
                    nc.tensor.matmul(psQ, sa_sb[:, mt * 4:(mt + 1) * 4], r1sq,
                                     start=(mt == 0), stop=(mt == MT - 1))

                outbuf = sb.tile([4, 2, L], F32)
                nc.scalar.activation(out=outbuf[:, 0, :], in_=psP,
                                     func=mybir.ActivationFunctionType.Copy)
                nc.vector.tensor_copy(outbuf[:, 1, :], psQ)
                nc.sync.dma_start(out=out[:, :, :], in_=outbuf)

            if loop:
                with tc.For_i(0, reps, 1):
                    body()
            else:
                for _ in range(reps):
                    body()
    _split_waits(nc)
    return nc


_NC = {}
LAST_IN_MAPS = None


def _get_nc(reps=1, loop=False):
    key = (reps, loop)
    if key not in _NC:
        _NC[key] = _build_nc(reps, loop)
    return _NC[key]


def _batchnorm_rows(x, gamma, beta):
    mu = x.mean(axis=0)
    var = x.var(axis=0)
    return (x - mu) / np.sqrt(var + EPS) * gamma + beta


def kernel(**inputs):
    f8 = np.float64
    H = np.asarray(inputs["H"], np.float32)
    Hd = H.astype(f8)
    S = Hd.sum(axis=0)                                   # colsum(H) [512]

    # ---------------- host: pool head -> assignments s ----------------
    pw_rel = inputs["pool_w_rel"].astype(f8)
    Wp = inputs["pool_w_root"].astype(f8) - pw_rel / DEG
    bp = inputs["pool_b_rel"].astype(f8) + (S @ pw_rel) / DEG
    rp = np.maximum(Hd @ Wp + bp, 0.0)                   # [2048, 2]
    xp = _batchnorm_rows(rp, inputs["pool_gamma"].astype(f8),
                         inputs["pool_beta"].astype(f8))
    zl = np.maximum(xp @ inputs["pool_w_lin"].astype(f8)
                    + inputs["pool_b_lin"].astype(f8), 0.0)
    zs = zl - zl.max(axis=-1, keepdims=True)
    es = np.exp(zs)
    s = es / es.sum(axis=-1, keepdims=True)              # [2048, 2]

    cs = s.sum(axis=0)                                   # [2]
    G = s.T @ s                                          # [2, 2]
    ent = float((-s * np.log(s + 1e-15)).sum(axis=-1).mean())
    # ||(ones - eye) - s s^T||_F^2 without materializing NxN:
    fro2 = N * N - N - 2.0 * float(cs @ cs) + 2.0 * float(np.trace(G)) \
        + float((G * G).sum())
    l1 = float(np.sqrt(max(fro2, 0.0)) / (N * N))

    # ---------------- device: embed1 + diffpool contraction ----------------
    e_rel = inputs["embed1_w_rel"].astype(f8)
    W1 = (inputs["embed1_w_root"].astype(f8) - e_rel / DEG).astype(np.float32)
    b1f = (inputs["embed1_b_rel"].astype(f8) + (S @ e_rel) / DEG).astype(np.float32)
    bop = np.zeros((1, 128 + L), ml_dtypes.bfloat16)
    bop[0, :128] = 1.0
    bop[0, 128:] = b1f

    HT = np.ascontiguousarray(H.T)                       # [512, 2048]
    sa = np.zeros((N, 4), np.float32)
    sa[:, 0:2] = s.astype(np.float32)
    sa[:, 2] = 1.0
    W1k = W1.reshape(KT, 128, L)                         # [4,128,512]

    in_maps = []
    for i in range(NCORES):
        hts = HT[:, i * M:(i + 1) * M].reshape(KT, 128, M)
        hwp = np.ascontiguousarray(
            np.concatenate([hts, W1k], axis=2).transpose(1, 0, 2)
            .reshape(128, KT * (M + L))).astype(ml_dtypes.bfloat16)
        sap = sa[i * M:(i + 1) * M].reshape(MT, 128, 4)
        sap = np.ascontiguousarray(
            sap.transpose(1, 0, 2).reshape(128, MT * 4)).astype(ml_dtypes.bfloat16)
        in_maps.append({"hw": hwp, "sab": sap, "bo": bop})

    global LAST_IN_MAPS
    LAST_IN_MAPS = in_maps
    res = run_bass_kernel_spmd(_get_nc(), in_maps, list(range(NCORES)))
    parts = np.stack([res.results[i]["out"] for i in range(NCORES)]).astype(f8)
    tot = parts.sum(axis=0)                              # [4, 2, 512]
    P = tot[0:2, 0, :]                                   # s^T @ r1
    mu1 = tot[2, 0, :] / N
    var1 = tot[2, 1, :] / N - mu1 * mu1
    a1 = inputs["embed1_gamma"].astype(f8) / np.sqrt(var1 + EPS)
    c1 = inputs["embed1_beta"].astype(f8) - a1 * mu1
    x = P * a1[None, :] + cs[:, None] * c1[None, :]      # pooled features [2,512]
    adj = np.outer(cs, cs) - G                           # pooled adjacency [2,2]

    # ---------------- host: embed3 + classifier ----------------
    deg3 = np.clip(adj.sum(axis=-1, keepdims=True), 1.0, None)
    agg = (adj @ x) / deg3
    z3 = agg @ inputs["embed3_w_rel"].astype(f8) + inputs["embed3_b_rel"].astype(f8) \
        + x @ inputs["embed3_w_root"].astype(f8)
    x3 = _batchnorm_rows(np.maximum(z3, 0.0), inputs["embed3_gamma"].astype(f8),
                         inputs["embed3_beta"].astype(f8))
    xm = x3.mean(axis=0)                                 # [512]
    h1 = np.maximum(xm @ inputs["w1"].astype(f8) + inputs["b1"].astype(f8), 0.0)
    z = h1 @ inputs["w2"].astype(f8) + inputs["b2"].astype(f8)
    zst = z - z.max()
    lp = zst - np.log(np.exp(zst).sum())
    return (lp.astype(np.float32).reshape(1, 2),
            np.float32(l1), np.float32(ent))


# revision 59
# speedup vs baseline: 1.0123x; 1.0047x over previous
"""Bass/Trainium2 kernel for nn_Net_58832462020811 (DiffPool GNN on a radius graph).

Key algebra (exact for this problem's inputs): the radius threshold (5e6) vastly
exceeds any pairwise distance of the N(0,1) inputs, so adj == ones - eye and
adj @ x == colsum(x) - x with degree 2047. Every DenseSAGEConv therefore
collapses to a single dense matmul with folded weights/bias:
    dense_sage(x) = x @ (w_root - w_rel/2047) + [b_rel + (colsum(x) @ w_rel)/2047]
The only heavy compute left is embed1: r1 = relu(H @ W1f + b1f) with
H [2048,512], W1f [512,512], followed by BatchNorm (train-mode stats over the
2048 rows) and the DiffPool contraction P = s^T @ r1 with s [2048,2].

Device work (8 cores, row-sharded 256 rows/core, zero collectives):
  z = H_rows @ W1f + b1f ; r1 = relu(z) ; per-core partials of
  [P0, P1, colsum(r1), colsum(r1^2)] -> out [4,512] per core.
All matmul operands are bf16 (single-pass PE, FWL-eligible; f32 PSUM
accumulate); measured end-to-end rel err is unchanged vs all-f32.
BatchNorm is applied after the fact via the folded form
  s^T @ BN(r1) = (s^T @ r1) * a + colsum(s) (x) (beta - a*mu),  a = gamma/rstd,
so no cross-core stat exchange is needed on device.

Host work (tiny, O(N*C) with C=2): the pool head (H @ Wpf is 2M MACs), softmax
assignments s, link/entropy losses, the pooled 2x2 graph, embed3 on [2,512],
and the final MLP + log_softmax.
"""

import ml_dtypes
import numpy as np

import concourse.bass as bass
import concourse.tile as tile
from concourse import mybir
from concourse.bass_utils import run_bass_kernel_spmd

N = 2048
L = 512
NCORES = 8
M = N // NCORES          # rows per core
DEG = float(N - 1)       # 2047, degree of every node
EPS = 1e-5

F32 = mybir.dt.float32
BF16 = mybir.dt.bfloat16


KT = L // 128   # 4 k-tiles
MT = M // 128   # 2 row-tiles per core
# Inputs (per core, bf16 matmul path):
#   hw  [128, KT*(M+L)]: chunk k at [:, k*(M+L):...]; within a chunk,
#                        cols 0:M = H_rows^T k-tile (lhsT), M:M+L = W1f k-tile
#   sab [128, MT*4]:    sab[p, t*4+j] = sa[t*128+p, j], sa cols [s0, s1, 1, 0]
#   bo  [1, 128+L]:     ones[128] ++ b1f[512]  (rank-1 bias matmul operands)


def _split_waits(nc, cap=1):
    """walrus codegen in this env rejects >1 sync wait per instruction
    ("Too many sync wait commands"); hoist extras onto same-engine NoOps
    placed immediately before (sequencer order makes this equivalent)."""
    for blk in nc.main_func.blocks:
        lst = blk.instructions
        for ins in list(lst):
            si = ins.sync_info
            if si is None or len(si.on_wait) <= cap:
                continue
            if ins.engine == mybir.EngineType.Unassigned:
                continue
            extra = list(si.on_wait[cap:])
            keep = list(si.on_wait[:cap])
            idx = lst.index(ins)
            for w in extra:
                nop = mybir.InstNoOp(
                    name=nc.get_next_instruction_name(),
                    ins=[], outs=[],
                    sync_info=mybir.SyncInfo(on_wait=[w], on_update=[]),
                )
                nop.engine = ins.engine
                lst.insert(idx, nop)
                idx += 1
            ins.sync_info = mybir.SyncInfo(on_wait=keep,
                                           on_update=list(si.on_update))


def _build_nc(reps=1, loop=False):
    nc = bass.Bass()
    hw = nc.dram_tensor("hw", [128, KT * (M + L)], BF16, kind="ExternalInput")
    sab = nc.dram_tensor("sab", [128, MT * 4], BF16, kind="ExternalInput")
    bo = nc.dram_tensor("bo", [1, 128 + L], BF16, kind="ExternalInput")
    out = nc.dram_tensor("out", [4, 2, L], F32, kind="ExternalOutput")

    with tile.TileContext(nc) as tc:
        with (
            tc.tile_pool(name="sb", bufs=1) as sb,
            tc.tile_pool(name="act", bufs=2) as act,
            tc.tile_pool(name="psz", bufs=2, space="PSUM") as psz_pool,
            tc.tile_pool(name="pacc", bufs=1, space="PSUM") as pacc,
        ):
            def body():
                big = sb.tile([128, KT * (M + L)], BF16)
                sa_sb = sb.tile([128, MT * 4], BF16)
                bo_sb = sb.tile([1, 128 + L], BF16)
                # small operands first (bias mm unblocks immediately); hw split
                # across two DGE queues (partition halves -> parallel SDMA) and
                # chunk0/rest so the first z-matmuls overlap the DMA tail
                nc.sync.dma_start(out=bo_sb, in_=bo[:, :])
                nc.sync.dma_start(out=sa_sb, in_=sab[:, :])
                c0 = M + L
                nc.sync.dma_start(out=big[0:64, 0:c0], in_=hw[0:64, 0:c0])
                nc.scalar.dma_start(out=big[64:128, 0:c0], in_=hw[64:128, 0:c0])
                nc.sync.dma_start(out=big[0:64, c0:], in_=hw[0:64, c0:])
                nc.scalar.dma_start(out=big[64:128, c0:], in_=hw[64:128, c0:])
                ck = [big[:, k * (M + L):(k + 1) * (M + L)] for k in range(KT)]

                psP = pacc.tile([4, L], F32)   # rows: P0, P1, colsum(r1), 0
                psQ = pacc.tile([4, L], F32)   # rows: *, *, colsum(r1^2), 0

                for mt in range(MT):
                    psz = psz_pool.tile([128, L], F32)
                    # bias via rank-1 matmul: ones[128,1] @ b1f[1,512]
                    nc.tensor.matmul(psz, bo_sb[0:1, 0:128],
                                     bo_sb[0:1, 128:128 + L],
                                     start=True, stop=False)
                    for k in range(KT):
                        nc.tensor.matmul(
                            psz,
                            ck[k][:, mt * 128:(mt + 1) * 128],
                            ck[k][:, M:M + L],
                            start=False,
                            stop=(k == KT - 1),
                        )
                    r1 = act.tile([128, L], BF16)
                    nc.scalar.activation(out=r1, in_=psz,
                                         func=mybir.ActivationFunctionType.Relu)
                    r1sq = act.tile([128, L], BF16)
                    nc.vector.tensor_mul(r1sq, r1, r1)
                    nc.tensor.matmul(psP, sa_sb[:, mt * 4:(mt + 1) * 4], r1,
                                     start=(mt == 0), stop=(mt == MT - 1))
                    nc.tensor.matmul(psQ, sa_sb[:, mt * 4:(mt + 1) * 4], r1sq,
                                     start=(mt == 0), stop=(mt == MT - 1))

                outbuf = sb.tile([4, 2, L], F32)
                nc.scalar.activation(out=outbuf[:, 0, :], in_=psP,
                                     func=mybir.ActivationFunctionType.Copy)
                nc.vector.tensor_copy(outbuf[:, 1, :], psQ)
                nc.sync.dma_start(out=out[:, :, :], in_=outbuf)

            if loop:
                with tc.For_i(0, reps, 1):
                    body()
            else:
                for _ in range(reps):
                    body()
    _split_waits(nc)
    return nc


_NC = {}
LAST_IN_MAPS = None


def _get_nc(reps=1, loop=False):
    key = (reps, loop)
    if key not in _NC:
        _NC[key] = _build_nc(reps, loop)
    return _NC[key]


def _batchnorm_rows(x, gamma, beta):
    mu = x.mean(axis=0)
    var = x.var(axis=0)
    return (x - mu) / np.sqrt(var + EPS) * gamma + beta


def kernel(**inputs):
    f8 = np.float64
    H = np.asarray(inputs["H"], np.float32)
    Hd = H.astype(f8)
    S = Hd.sum(axis=0)                                   # colsum(H) [512]

    # ---------------- host: pool head -> assignments s ----------------
    pw_rel = inputs["pool_w_rel"].astype(f8)
    Wp = inputs["pool_w_root"].astype(f8) - pw_rel / DEG
    bp = inputs["pool_b_rel"].astype(f8) + (S @ pw_rel) / DEG
    rp = np.maximum(Hd @ Wp + bp, 0.0)                   # [2048, 2]
    xp = _batchnorm_rows(rp, inputs["pool_gamma"].astype(f8),
                         inputs["pool_beta"].astype(f8))
    zl = np.maximum(xp @ inputs["pool_w_lin"].astype(f8)
                    + inputs["pool_b_lin"].astype(f8), 0.0)
    zs = zl - zl.max(axis=-1, keepdims=True)
    es = np.exp(zs)
    s = es / es.sum(axis=-1, keepdims=True)              # [2048, 2]

    cs = s.sum(axis=0)                                   # [2]
    G = s.T @ s                                          # [2, 2]
    ent = float((-s * np.log(s + 1e-15)).sum(axis=-1).mean())
    # ||(ones - eye) - s s^T||_F^2 without materializing NxN:
    fro2 = N * N - N - 2.0 * float(cs @ cs) + 2.0 * float(np.trace(G)) \
        + float((G * G).sum())
    l1 = float(np.sqrt(max(fro2, 0.0)) / (N * N))

    # ---------------- device: embed1 + diffpool contraction ----------------
    e_rel = inputs["embed1_w_rel"].astype(f8)
    W1 = (inputs["embed1_w_root"].astype(f8) - e_rel / DEG).astype(np.float32)
    b1f = (inputs["embed1_b_rel"].astype(f8) + (S @ e_rel) / DEG).astype(np.float32)
    bop = np.zeros((1, 128 + L), ml_dtypes.bfloat16)
    bop[0, :128] = 1.0
    bop[0, 128:] = b1f

    HT = np.ascontiguousarray(H.T)                       # [512, 2048]
    sa = np.zeros((N, 4), np.float32)
    sa[:, 0:2] = s.astype(np.float32)
    sa[:, 2] = 1.0
    W1k = W1.reshape(KT, 128, L)                         # [4,128,512]

    in_maps = []
    for i in range(NCORES):
        hts = HT[:, i * M:(i + 1) * M].reshape(KT, 128, M)
        hwp = np.ascontiguousarray(
            np.concatenate([hts, W1k], axis=2).transpose(1, 0, 2)
            .reshape(128, KT * (M + L))).astype(ml_dtypes.bfloat16)
        sap = sa[i * M:(i + 1) * M].reshape(MT, 128, 4)
        sap = np.ascontiguousarray(
            sap.transpose(1, 0, 2).reshape(128, MT * 4)).astype(ml_dtypes.bfloat16)
        in_maps.append({"hw": hwp, "sab": sap, "bo": bop})

    global LAST_IN_MAPS
    LAST_IN_MAPS = in_maps
    res = run_bass_kernel_spmd(_get_nc(), in_maps, list(range(NCORES)))
    parts = np.stack([res.results[i]["out"] for i in range(NCORES)]).astype(f8)
    tot = parts.sum(axis=0)                              # [4, 2, 512]
    P = tot[0:2, 0, :]                                   # s^T @ r1
    mu1 = tot[2, 0, :] / N
    var1 = tot[2, 1, :] / N - mu1 * mu1
    a1 = inputs["embed1_gamma"].astype(f8) / np.sqrt(var1 + EPS)
    c1 = inputs["embed1_beta"].astype(f8) - a1 * mu1
    x = P * a1[None, :] + cs[:, None] * c1[None, :]      # pooled features [2,512]
    adj = np.outer(cs, cs) - G                           # pooled adjacency [2,2]

    # ---------------- host: embed3 + classifier ----------------
    deg3 = np.clip(adj.sum(axis=-1, keepdims=True), 1.0, None)
    agg = (adj @ x) / deg3
    z3 = agg @ inputs["embed3_w_rel"].astype(f8) + inputs["embed3_b_rel"].astype(f8) \
        + x @ inputs["embed3_w_root"].astype(f8)
    x3 = _batchnorm_rows(np.maximum(z3, 0.0), inputs["embed3_gamma"].astype(f8),
                         inputs["embed3_beta"].astype(f8))
    xm = x3.mean(axis=0)                                 # [512]
    h1 = np.maximum(xm @ inputs["w1"].astype(f8) + inputs["b1"].astype(f8), 0.0)
    z = h1 @ inputs["w2"].astype(f8) + inputs["b2"].astype(f8)
    zst = z - z.max()
    lp = zst - np.log(np.exp(zst).sum())
    return (lp.astype(np.float32).reshape(1, 2),
            np.float32(l1), np.float32(ent))
